# revision 1
# baseline (speedup 1.0000x reference)
"""Bilinear 2x upsample (8,256,256,32) f32 -> (8,512,512,32) on 8 TRN2 cores.

Strategy (data-parallel over batch N=8, one sample per core):
  The op is a separable 2x bilinear upsample with fixed tap weights
  {0.25, 0.75} (half-pixel centers, scale 0.5), plus clamped edges.

  Per core:
   - Vertical pass on TensorE: tmp = Wv.T @ x, where Wv is the (256 -> 512)
     bidiagonal interpolation matrix (host-precomputed, fp32, exact edge
     handling baked in). x rows live on partitions, so the y-contraction is
     a natural matmul. Full K=128 matmuls, output rows in chunks of 128.
   - Horizontal pass on ScalarE+VectorE: ScalarE evacuates PSUM with the
     two x-tap scales folded in (A = 0.25*tmp, B = 0.75*tmp); VectorE adds
     shifted views: out[2j] = A[j-1] + B[j], out[2j+1] = B[j] + A[j+1]
     (x-shift = 32 fp32 channel elements in the free dim), writing the
     even/odd results interleaved so output DMA is fully contiguous.
  Everything is fp32 end to end.
"""

import numpy as np

import concourse.bass as bass
import concourse.mybir as mybir
from concourse import bacc
from concourse.tile import TileContext
from concourse.bass_utils import run_bass_kernel_spmd

N, H, W, C = 8, 256, 256, 32
OH, OW = 512, 512
FREE = W * C       # 8192 input row elements
OFREE = OW * C     # 16384 output row elements
G = C              # one x-group = 32 elements
NCORES = 8

F32 = mybir.dt.float32


def _build_wv() -> np.ndarray:
    """[256, 512] fp32 vertical weights, replicating the reference exactly."""
    oy = np.arange(OH, dtype=np.float32)
    gy = np.maximum((oy + np.float32(0.5)) * np.float32(H / OH) - np.float32(0.5),
                    np.float32(0.0)).astype(np.float32)
    y0 = np.floor(gy).astype(np.int32)
    y1 = y0 + (y0 < H - 1).astype(np.int32)
    h0 = (gy - y0.astype(np.float32)).astype(np.float32)
    wv = np.zeros((H, OH), np.float32)
    # np.add.at to handle y0 == y1 at the clamped top edge (weights sum to 1)
    np.add.at(wv, (y0, np.arange(OH)), (np.float32(1.0) - h0))
    np.add.at(wv, (y1, np.arange(OH)), h0)
    return wv


_PROGRAM_CACHE = {}
# Dev knob: "full" | "dma" (input+output DMA only) | "mm" (input DMA + matmuls)
# | "mmact" (adds PSUM evacuation). Used for on-HW bottleneck attribution.
VARIANT = "full"


def _build_program(n_reps: int = 1) -> bass.Bass:
    """n_reps > 1 repeats the whole pipeline (including the input DMA)
    inside one NEFF, for steady-state HW timing; output is identical."""
    key = (n_reps, VARIANT)
    if key in _PROGRAM_CACHE:
        return _PROGRAM_CACHE[key]

    nc = bacc.Bacc("TRN2", target_bir_lowering=False, debug=False)
    # One packed input: [x_rows_0_127 | x_rows_128_255 | wv_0_127 | wv_128_255]
    # along the free dim, so a single DMA (single DMA semaphore) provides all
    # matmul operands: the HW weight-load slot only budgets one sync wait.
    xw = nc.dram_tensor("xw", [128, 2 * FREE + 2 * OH], F32, kind="ExternalInput")
    y = nc.dram_tensor("y", [OH, OFREE], F32, kind="ExternalOutput")

    with TileContext(nc) as tc:
        with (
            tc.tile_pool(name="xin", bufs=1) as xpool,
            tc.tile_pool(name="abuf", bufs=2) as apool,
            tc.tile_pool(name="bbuf", bufs=2) as bpool,
            tc.tile_pool(name="obuf", bufs=2) as opool,
            tc.tile_pool(name="ps", bufs=8, space="PSUM") as pspool,
        ):
          for rep in range(n_reps):
            # Packed layout: [wv halves (1024) | x rows 0-127 (8192) | x rows
            # 128-255 (8192)]. Two DMAs: weights + first x half, then the
            # second x half, so chunk 0's matmuls start after ~half the input
            # landed. Chunk order (0,3,1,2) keeps every matmul at <=1 sync
            # wait: the first MM of chunk 0 waits on DMA1, the first MM of
            # chunk 3 on DMA2, and chunks 1/2 (which read both halves) see
            # already-satisfied clocks.
            xw_t = xpool.tile([128, 2 * FREE + 2 * OH], F32, tag="xw",
                              name=f"xw_{rep}")
            # Piece-wise input stream (0.5 MiB weights + 8 x 1 MiB x-pieces):
            # chunk 0's first matmul only needs the first piece, and on rep
            # boundaries each piece can reload as soon as its readers are
            # done. Multi-wait matmuls are legalized by Bacc's event-sem pass.
            nc.sync.dma_start(out=xw_t[:, 0:2 * OH], in_=xw[:, 0:2 * OH])
            for piece in range(8):
                o = 2 * OH + 2048 * piece
                nc.sync.dma_start(out=xw_t[:, o:o + 2048], in_=xw[:, o:o + 2048])
            w2 = xw_t[:, 0:2 * OH]
            x2 = xw_t[:, 2 * OH:2 * OH + 2 * FREE]

            # Which (weight-half, input-half) pairs contribute to each
            # 128-row chunk: chunk m covers oy in [128m, 128m+128) and needs
            # img rows [64m-1, 64m+64].
            chunk_srcs = [[0], [0, 1], [0, 1], [1]]

            for m in (0, 1, 2, 3):
                srcs = chunk_srcs[m]
                bufA = [None, None]
                bufB = [None, None]
                for h in range(2):
                    bufA[h] = apool.tile([128, 4096], F32, tag="A", name=f"bufA_{rep}_{m}_{h}")
                    bufB[h] = bpool.tile([128, 4096], F32, tag="B", name=f"bufB_{rep}_{m}_{h}")

                for h in range(2):
                    A, B = bufA[h], bufB[h]
                    for pt in range(2):
                        pss = [pspool.tile([128, 512], F32, tag="ps", name=f"ps_{rep}_{m}_{h}_{pt}_{s}") for s in range(4)]
                        for s in range(4):
                            ps = pss[s]
                            nt = 8 * h + 4 * pt + s
                            for idx, a in enumerate(srcs):
                                if VARIANT == "dma":
                                    continue
                                nc.tensor.matmul(
                                    out=ps[:, :],
                                    lhsT=w2[:, a * OH + 128 * m:a * OH + 128 * m + 128],
                                    rhs=x2[:, a * FREE + 512 * nt:a * FREE + 512 * nt + 512],
                                    start=(idx == 0),
                                    stop=(idx == len(srcs) - 1),
                                )
                        # Horizontal tap scales folded into PSUM evacuation.
                        # A = 0.25*tmp is exact (exponent shift), so
                        # B = 3*A == round(0.75*tmp) bit-exactly; computing
                        # half the B tiles on VectorE offloads ScalarE.
                        for s in range(4):
                            if VARIANT in ("dma", "mm"):
                                continue
                            o = 2048 * pt + 512 * s
                            nc.scalar.mul(A[:, o:o + 512], pss[s][:, :], 0.25)
                            if h == 0:
                                nc.scalar.mul(B[:, o:o + 512], pss[s][:, :], 0.75)
                            else:
                                nc.vector.tensor_scalar_mul(
                                    B[:, o:o + 512], A[:, o:o + 512], 3.0
                                )

                for h in range(2):
                    A, B = bufA[h], bufB[h]
                    # One [128, 8192] out tile per half: 4 MiB output DMAs.
                    ot = opool.tile([128, 8192], F32, tag="out", name=f"ot_{rep}_{m}_{h}")
                    v = ot[:, :].rearrange("p (j t c) -> p j t c", t=2, c=G)
                    do_tt = VARIANT == "full"
                    if not do_tt:
                        # stripped variants: touch the tile so Tile allocates
                        # it for the output DMA read
                        nc.vector.memset(ot[:, 0:1], 0.0)

                    def g3(ap):
                        return ap.rearrange("p (j c) -> p j c", c=G)

                    # even pairs 1..127: A[j-1] + B[j]
                    if do_tt:
                      nc.vector.tensor_add(
                        out=v[:, 1:128, 0, :],
                        in0=g3(A[:, 0:4064]),
                        in1=g3(B[:, 32:4096]),
                      )
                    # even pair 0: A[-1] + B[0]
                    a_prev = bufA[0][:, 0:32] if h == 0 else bufA[0][:, 4064:4096]
                    if do_tt:
                      nc.vector.tensor_add(
                        out=v[:, 0:1, 0, :],
                        in0=g3(a_prev),
                        in1=g3(B[:, 0:32]),
                      )
                    # odd pairs 0..126: B[j] + A[j+1]
                    if do_tt:
                      nc.vector.tensor_add(
                        out=v[:, 0:127, 1, :],
                        in0=g3(B[:, 0:4064]),
                        in1=g3(A[:, 32:4096]),
                      )
                    # odd pair 127: B[127] + A[128]
                    a_next = bufA[1][:, 0:32] if h == 0 else bufA[1][:, 4064:4096]
                    if do_tt:
                      nc.vector.tensor_add(
                        out=v[:, 127:128, 1, :],
                        in0=g3(B[:, 4064:4096]),
                        in1=g3(a_next),
                      )
                    # Output DMAs ride the SWDGE (gpsimd) path so the SP
                    # HWDGE ring stays free for input prefetch.
                    dma_eng = nc.gpsimd
                    dma_eng.dma_start(
                        out=y[128 * m:128 * m + 128, 8192 * h:8192 * h + 8192],
                        in_=ot[:, :],
                    )

    # Legalize for TRN2's 1-wait-per-instruction limit (event-semaphore
    # splitting), register allocation, etc.
    nc.compile()

    _PROGRAM_CACHE[key] = nc
    return nc


def pack_input(sample: np.ndarray, wv: np.ndarray) -> np.ndarray:
    """[128, 2*OH + 2*FREE]: wv halves | x rows 0-127 | x rows 128-255."""
    xr = sample.reshape(H, FREE)
    return np.concatenate(
        [wv[0:128], wv[128:256], xr[0:128], xr[128:256]], axis=1
    ).astype(np.float32)


def kernel(img: np.ndarray) -> np.ndarray:
    assert img.shape == (N, H, W, C), img.shape
    img = np.ascontiguousarray(img, dtype=np.float32)
    wv = _build_wv()
    nc = _build_program()
    in_maps = [{"xw": pack_input(img[i], wv)} for i in range(NCORES)]
    res = run_bass_kernel_spmd(nc, in_maps, core_ids=list(range(NCORES)))
    out = np.stack([r["y"].reshape(OH, OW, C) for r in res.results], axis=0)
    return out


if __name__ == "__main__":
    rng = np.random.default_rng(0)
    img = rng.standard_normal((N, H, W, C), dtype=np.float32)
    out = kernel(img)
    print(out.shape, out.dtype)



# revision 2
# speedup vs baseline: 2.0422x; 2.0422x over previous
"""Bilinear 2x upsample (8,256,256,32) f32 -> (8,512,512,32) on 8 TRN2 cores.

Strategy (data-parallel over batch N=8, one sample per core):
  The op is a separable 2x bilinear upsample with fixed tap weights
  {0.25, 0.75} (half-pixel centers, scale 0.5), plus clamped edges.
  The kernel is HBM-DMA-bound, so all HBM I/O is fp16 (the 2e-2 rel-err
  gate leaves ~13x margin for fp16's 2^-11 rounding), halving traffic
  vs fp32: 4.5 MB in + 16.8 MB out per core.

  Per core:
   - Vertical pass on TensorE (fp16 in, fp32 PSUM accumulate):
     A = (0.25*Wv).T @ x, where Wv is the (256 -> 512) bidiagonal
     interpolation matrix (host-precomputed; the 0.25 horizontal tap is
     folded in, all weights {1/16, 3/16, 1/4} exact in fp16).
   - ScalarE evacuates PSUM -> fp16 into a PADDED A tile (one extra
     x-group on each side, filled with the edge-clamped duplicate), so
     the horizontal pass needs no per-edge fixup ops.
   - Horizontal pass on VectorE: B = 3*A (tensor_scalar, 4x DVE mode for
     packed fp16), then out_even[j] = B[j] + A[j-1], out_odd[j] = B[j] +
     A[j+1] as two shifted tensor_adds (2x DVE mode) writing the
     even/odd results interleaved so output DMA is fully contiguous.
   - Output DMAs (fp16) ride the SWDGE (gpsimd) path so the SP HWDGE
     ring stays free for input prefetch.
"""

import numpy as np

import concourse.bass as bass
import concourse.mybir as mybir
from concourse import bacc
from concourse.tile import TileContext
from concourse.bass_utils import run_bass_kernel_spmd

N, H, W, C = 8, 256, 256, 32
OH, OW = 512, 512
FREE = W * C       # 8192 input row elements
OFREE = OW * C     # 16384 output row elements
G = C              # one x-group = 32 elements
NCORES = 8

F32 = mybir.dt.float32
F16 = mybir.dt.float16


def _build_wv() -> np.ndarray:
    """[256, 512] fp32 vertical weights, replicating the reference exactly."""
    oy = np.arange(OH, dtype=np.float32)
    gy = np.maximum((oy + np.float32(0.5)) * np.float32(H / OH) - np.float32(0.5),
                    np.float32(0.0)).astype(np.float32)
    y0 = np.floor(gy).astype(np.int32)
    y1 = y0 + (y0 < H - 1).astype(np.int32)
    h0 = (gy - y0.astype(np.float32)).astype(np.float32)
    wv = np.zeros((H, OH), np.float32)
    # np.add.at to handle y0 == y1 at the clamped top edge (weights sum to 1)
    np.add.at(wv, (y0, np.arange(OH)), (np.float32(1.0) - h0))
    np.add.at(wv, (y1, np.arange(OH)), h0)
    return wv


_PROGRAM_CACHE = {}
# Dev knob: "full" | "dma" (input+output DMA only) | "mm" (input DMA + matmuls)
# | "mmact" (adds PSUM evacuation). Used for on-HW bottleneck attribution.
VARIANT = "full"


def _build_program(n_reps: int = 1) -> bass.Bass:
    """n_reps > 1 repeats the whole pipeline (including the input DMA)
    inside one NEFF, for steady-state HW timing; output is identical."""
    key = (n_reps, VARIANT)
    if key in _PROGRAM_CACHE:
        return _PROGRAM_CACHE[key]

    nc = bacc.Bacc("TRN2", target_bir_lowering=False, debug=False)
    # One packed fp16 input: [wv half0 | wv half1 | x rows 0-127 | x rows
    # 128-255] along the free dim.
    xw = nc.dram_tensor("xw", [128, 2 * OH + 2 * FREE], F16, kind="ExternalInput")
    y = nc.dram_tensor("y", [OH, OFREE], F16, kind="ExternalOutput")

    with TileContext(nc) as tc:
        with (
            tc.tile_pool(name="xin", bufs=1) as xpool,
            tc.tile_pool(name="abuf", bufs=2) as apool,
            tc.tile_pool(name="bbuf", bufs=2) as bpool,
            tc.tile_pool(name="obuf", bufs=3) as opool,
            tc.tile_pool(name="ps", bufs=8, space="PSUM") as pspool,
        ):
          for rep in range(n_reps):
            xw_t = xpool.tile([128, 2 * OH + 2 * FREE], F16, tag="xw",
                              name=f"xw_{rep}")
            # Piece-wise input stream (0.25 MB weights + 8 x 0.5 MB x-pieces):
            # chunk 0's first matmul only needs the first piece, and on rep
            # boundaries each piece can reload as soon as its readers are
            # done.
            nc.sync.dma_start(out=xw_t[:, 0:2 * OH], in_=xw[:, 0:2 * OH])
            for piece in range(8):
                o = 2 * OH + 2048 * piece
                nc.sync.dma_start(out=xw_t[:, o:o + 2048], in_=xw[:, o:o + 2048])
            w2 = xw_t[:, 0:2 * OH]
            x2 = xw_t[:, 2 * OH:2 * OH + 2 * FREE]

            # Which (weight-half, input-half) pairs contribute to each
            # 128-row chunk: chunk m covers oy in [128m, 128m+128) and needs
            # img rows [64m-1, 64m+64].
            chunk_srcs = [[0], [0, 1], [0, 1], [1]]

            for m in (0, 1, 2, 3):
                srcs = chunk_srcs[m]
                # Padded A: groups [-1, 256] of A[j] = 0.25*tmp[j]; pads hold
                # the x-edge clamp duplicates (A[-1]:=A[0], A[256]:=A[255]).
                ap = apool.tile([128, 2 * G + FREE], F16, tag="A",
                                name=f"apad_{rep}_{m}")
                bt = bpool.tile([128, FREE], F16, tag="B", name=f"b_{rep}_{m}")
                for t in range(16):
                    ps = pspool.tile([128, 512], F32, tag="ps",
                                     name=f"ps_{rep}_{m}_{t}")
                    for idx, a in enumerate(srcs):
                        if VARIANT == "dma":
                            continue
                        nc.tensor.matmul(
                            out=ps[:, :],
                            lhsT=w2[:, a * OH + 128 * m:a * OH + 128 * m + 128],
                            rhs=x2[:, a * FREE + 512 * t:a * FREE + 512 * t + 512],
                            start=(idx == 0),
                            stop=(idx == len(srcs) - 1),
                        )
                    if VARIANT in ("dma", "mm"):
                        continue
                    # PSUM -> padded fp16 A (dtype-converting copy).
                    o = G + 512 * t
                    nc.scalar.copy(ap[:, o:o + 512], ps[:, :])
                    if t == 0:
                        nc.scalar.copy(ap[:, 0:G], ps[:, 0:G])
                    if t == 15:
                        nc.scalar.copy(ap[:, G + FREE:2 * G + FREE],
                                       ps[:, 512 - G:512])

                do_tt = VARIANT == "full"
                if do_tt:
                    # B = 3*A (exact: A has 11-bit significand, 3*A needs 13,
                    # DVE computes in fp32 and rounds once). 4x DVE mode.
                    nc.vector.tensor_scalar_mul(bt[:, :], ap[:, G:G + FREE], 3.0)

                a3 = ap[:, :].rearrange("p (j c) -> p j c", c=G)
                b3 = bt[:, :].rearrange("p (j c) -> p j c", c=G)
                for h in range(2):
                    # One [128, 8192] out tile per half: 2 MB output DMAs.
                    ot = opool.tile([128, FREE], F16, tag="out",
                                    name=f"ot_{rep}_{m}_{h}")
                    v = ot[:, :].rearrange("p (j t c) -> p j t c", t=2, c=G)
                    if not do_tt:
                        # stripped variants: touch the tile so Tile allocates
                        # it for the output DMA read
                        nc.vector.memset(ot[:, 0:1], 0.0)
                    jb = 128 * h
                    if do_tt:
                        # even j: B[j] + A[j-1]; Apad group index of A[j-1]
                        # is j, so the window starts at jb.
                        nc.vector.tensor_add(
                            out=v[:, :, 0, :],
                            in0=b3[:, jb:jb + 128, :],
                            in1=a3[:, jb:jb + 128, :],
                        )
                        # odd j: B[j] + A[j+1]; Apad group of A[j+1] is j+2.
                        nc.vector.tensor_add(
                            out=v[:, :, 1, :],
                            in0=b3[:, jb:jb + 128, :],
                            in1=a3[:, jb + 2:jb + 130, :],
                        )
                    nc.gpsimd.dma_start(
                        out=y[128 * m:128 * m + 128,
                              FREE * h:FREE * h + FREE],
                        in_=ot[:, :],
                    )

    # Legalize for TRN2's 1-wait-per-instruction limit (event-semaphore
    # splitting), register allocation, etc.
    nc.compile()

    _PROGRAM_CACHE[key] = nc
    return nc


def pack_input(sample: np.ndarray, wv: np.ndarray) -> np.ndarray:
    """fp16 [128, 2*OH + 2*FREE]: 0.25*wv halves | x rows 0-127 | x 128-255."""
    xr = np.asarray(sample, np.float32).reshape(H, FREE)
    wq = (np.float32(0.25) * wv)  # {1/16, 3/16, 1/4}: exact in fp16
    return np.concatenate(
        [wq[0:128], wq[128:256], xr[0:128], xr[128:256]], axis=1
    ).astype(np.float16)


def kernel(img: np.ndarray) -> np.ndarray:
    assert img.shape == (N, H, W, C), img.shape
    img = np.ascontiguousarray(img, dtype=np.float32)
    wv = _build_wv()
    nc = _build_program()
    in_maps = [{"xw": pack_input(img[i], wv)} for i in range(NCORES)]
    res = run_bass_kernel_spmd(nc, in_maps, core_ids=list(range(NCORES)))
    out = np.stack(
        [np.asarray(r["y"], np.float32).reshape(OH, OW, C) for r in res.results],
        axis=0,
    )
    return out


if __name__ == "__main__":
    rng = np.random.default_rng(0)
    img = rng.standard_normal((N, H, W, C), dtype=np.float32)
    out = kernel(img)
    print(out.shape, out.dtype)
